# revision 31
# baseline (speedup 1.0000x reference)
"""Trainium2 Bass kernel for nn_AutoCorrelation (8 NeuronCores, data-parallel over batch).

Algorithm (reference: AutoCorrelation block):
  corr = irfft(rfft(q, L) * conj(rfft(k, L)))        # circular cross-correlation
  top-6 delays from batch-mean of corr (mean over H,E then N)
  out  = sum_k softmax(mean[:, idx])_k * roll(v, -idx_k)

Implementation (two launches, host does only the tiny (N,L) topk/softmax glue):
  - Phase 1: radix-2-real DFT as dense TensorE matmuls, PE-paced (96 matmuls
    per batch item, ~216ns each back-to-back). DVE does the radix-2
    butterflies, complex products and final u+/-w combines (all bf16 2x-mode
    tensor_tensor); ACT stages every PSUM->SBUF bf16 conversion and its
    accumulator collects sum_r u / sum_r w per tau, which IS the (N,L) topk
    mean statistic (host just forms (su+sw)/R, (su-sw)/R). GpSimd does no
    compute (concurrent DVE+Pool execution inflates both ~3x on HW). Inputs
    stream as paired-block DMAs spread over the 3 DMA queues, inverse-const
    load deferred so it can't gate the first matmul.
  - Phase 2: out = sum_k w*roll(v) as PSUM-accumulated matmuls with w-scaled
    shifted-identity stationaries; delay-0-style taps (remainder 0) fold into
    the PSUM->SBUF copy as a DVE scalar_tensor_tensor with a per-partition
    weight vector, removing 8 matmuls per batch item.
"""
import math
import sys

sys.path.insert(0, "/opt/trn_rl_repo")

import numpy as np
import ml_dtypes

import concourse.bass as bass
import concourse.tile as tile
from concourse import bacc, mybir
from concourse.bass import ts
from concourse.bass_utils import run_bass_kernel_spmd

_dt = mybir.dt

N, L, H, E = 32, 1024, 8, 64
R = H * E                 # 512 rows (h,e) per batch item
NCORES = 8
NLOC = N // NCORES        # 4 batch items per core
F2 = 256                  # freqs per radix-2 half (even / odd)
TOPK = int(1.0 * math.log(L))  # 6
LB = L // 128             # 8 l/tau blocks
HB = 4                    # 128-blocks per 512-half

TRACE = [False]           # test.py flips this to collect exec_time_ns
LAST_EXEC_NS = [0, 0]     # phase1, phase2 exec time (when TRACE)


def _dft_mats():
    """Radix-2 split matrices. Forward (contract over l' = 0..511):
    even freqs X[2m] = (x1+x2) @ [C5 | S5m] (S5m slot 0 = f=512 Nyquist),
    odd freqs X[2m+1] = (x1-x2) @ [Mre | Mim] (twiddle folded in).
    Inverse: u = Pe_re@Au + Pe_im@Bu, w = Po_re@Aw + Po_im@Bw,
    corr[t] = u+w, corr[t+512] = u-w."""
    l = np.arange(512)[:, None].astype(np.float64)
    m = np.arange(F2)[None, :].astype(np.float64)
    C5 = np.cos(2 * np.pi * l * m / 512)
    S5 = -np.sin(2 * np.pi * l * m / 512)
    S5[:, 0] = (-1.0) ** np.arange(512)
    Mre = np.cos(2 * np.pi * l * (2 * m + 1) / L)
    Mim = -np.sin(2 * np.pi * l * (2 * m + 1) / L)
    t = np.arange(512)[None, :].astype(np.float64)
    mm = np.arange(F2)[:, None].astype(np.float64)
    Au = (2.0 / L) * np.cos(2 * np.pi * mm * t / 512)
    Bu = -(2.0 / L) * np.sin(2 * np.pi * mm * t / 512)
    Au[0, :] = 1.0 / L
    Bu[0, :] = (1.0 / L) * ((-1.0) ** np.arange(512))
    Aw = (2.0 / L) * np.cos(2 * np.pi * t * (2 * mm + 1) / L)
    Bw = -(2.0 / L) * np.sin(2 * np.pi * t * (2 * mm + 1) / L)
    return C5, S5, Mre, Mim, Au, Bu, Aw, Bw


def _build_phase1():
    store = _dt.bfloat16

    nc = bacc.Bacc("TRN2", target_bir_lowering=False, debug=False,
                   num_devices=NCORES)
    q_d = nc.dram_tensor("q", [NLOC, L, R], store, kind="ExternalInput").ap()
    k_d = nc.dram_tensor("k", [NLOC, L, R], store, kind="ExternalInput").ap()
    # cf cols: c5 | s5 | mre | mim  (each [512, 256])
    cf_d = nc.dram_tensor("cf", [512, 4 * F2], store,
                          kind="ExternalInput").ap()
    # ci cols: au | bu | aw | bw  (each [256, 512])
    ci_d = nc.dram_tensor("ci", [F2, 4 * 512], store,
                          kind="ExternalInput").ap()
    corr_d = nc.dram_tensor("corr", [NLOC, L, R], store,
                            kind="ExternalOutput").ap()
    # stat: cols tb = sum_r u[tb*128+p, r], cols 4+tb = sum_r w[...]
    pacc_d = nc.dram_tensor("pacc", [NLOC, 128, 8], _dt.float32,
                            kind="ExternalOutput").ap()

    def mm(ps, lhsT, rhs, start, stop):
        nc.tensor.matmul(ps, lhsT, rhs, start=start, stop=stop)

    def rearr(ap):
        return ap.rearrange("(j p) r -> p j r", p=128)

    with tile.TileContext(nc) as tc:
        with tc.tile_pool(name="const", bufs=1) as cp, \
             tc.tile_pool(name="qk", bufs=2) as qk, \
             tc.tile_pool(name="ed", bufs=8) as edp, \
             tc.tile_pool(name="st", bufs=5) as stp, \
             tc.tile_pool(name="tmp", bufs=3) as tp, \
             tc.tile_pool(name="pp", bufs=10) as pp, \
             tc.tile_pool(name="uw", bufs=6) as uwp, \
             tc.tile_pool(name="out", bufs=2) as op, \
             tc.tile_pool(name="psq", bufs=1, space="PSUM") as psq, \
             tc.tile_pool(name="psk", bufs=1, space="PSUM") as psk, \
             tc.tile_pool(name="psi", bufs=2, space="PSUM") as psi:

            # ---- pipeline fill: 3 dma queues, need-ordered ----
            # sync:   cf_a, q0{0,4}, q0{2,6}, k1, q3 (+corr outs n=0,2)
            # scalar: cf_b, q0{1,5}, q0{3,7}, q2, k3 (+corr outs n=1,3)
            # gpsimd: k0 pairs x4, q1, ci, k2        (+pacc outs)
            cf_sb = cp.tile([128, HB, 4 * F2], store, tag="cf")
            cfr = cf_d[:, 0:2 * F2].rearrange("(j p) c -> p j c", p=128)
            nc.sync.dma_start(cf_sb[:, 0:2, 0:2 * F2], cfr[:, 0:2, :])
            nc.sync.dma_start(cf_sb[:, 2:4, 0:2 * F2], cfr[:, 2:4, :])
            cfr2 = cf_d[:, 2 * F2:4 * F2].rearrange("(j p) c -> p j c", p=128)
            nc.scalar.dma_start(cf_sb[:, 0:2, 2 * F2:4 * F2], cfr2[:, 0:2, :])
            nc.scalar.dma_start(cf_sb[:, 2:4, 2 * F2:4 * F2], cfr2[:, 2:4, :])

            qm_all, km_all = [None] * NLOC, [None] * NLOC
            t = qk.tile([128, LB, R], store, tag="q")
            qr = rearr(q_d[0])
            nc.sync.dma_start(t[:, 0:5:4, :], qr[:, 0:5:4, :])
            nc.scalar.dma_start(t[:, 1:6:4, :], qr[:, 1:6:4, :])
            nc.sync.dma_start(t[:, 2:7:4, :], qr[:, 2:7:4, :])
            nc.scalar.dma_start(t[:, 3:8:4, :], qr[:, 3:8:4, :])
            qm_all[0] = t
            t = qk.tile([128, LB, R], store, tag="k")
            kr = rearr(k_d[0])
            for j in range(HB):
                nc.gpsimd.dma_start(t[:, j:j + 5:4, :], kr[:, j:j + 5:4, :])
            km_all[0] = t

            t = qk.tile([128, LB, R], store, tag="q")
            nc.gpsimd.dma_start(t[:], rearr(q_d[1]))
            qm_all[1] = t
            t = qk.tile([128, LB, R], store, tag="k")
            nc.sync.dma_start(t[:], rearr(k_d[1]))
            km_all[1] = t

            ci_sb = cp.tile([128, 2, 4 * 512], store, tag="ci")
            nc.gpsimd.dma_start(ci_sb[:],
                                ci_d.rearrange("(g p) c -> p g c", p=128))
            if NLOC > 2:
                t = qk.tile([128, LB, R], store, tag="q")
                nc.scalar.dma_start(t[:], rearr(q_d[2]))
                qm_all[2] = t
                t = qk.tile([128, LB, R], store, tag="k")
                nc.gpsimd.dma_start(t[:], rearr(k_d[2]))
                km_all[2] = t
            if NLOC > 3:
                t = qk.tile([128, LB, R], store, tag="q")
                nc.sync.dma_start(t[:], rearr(q_d[3]))
                qm_all[3] = t
                t = qk.tile([128, LB, R], store, tag="k")
                nc.scalar.dma_start(t[:], rearr(k_d[3]))
                km_all[3] = t

            def fwd_st(name_i, j, mb):
                off = name_i * F2 + mb * 128
                return cf_sb[:, j, off:off + 128]

            def inv_st(name_i, gb, tb):
                off = name_i * 512 + tb * 128
                return ci_sb[:, gb, off:off + 128]

            state = [None] * NLOC  # per-n (pre_sb, pim_sb) for inverse
            accs = [None] * NLOC

            def forward(n):
                qm, km = qm_all[n], km_all[n]
                # radix-2 butterflies on DVE in PE-consumption order. For
                # batch 0 the even groups contract qm/km directly with
                # 8-deep PSUM chains (16 extra matmuls once) so the PE can
                # start as soon as q0 lands instead of waiting on DVE.
                direct = (n == 0)
                eq, dq, ek, dk = [], [], [], []
                plan = [("dq", dq, qm, "tensor_sub"),
                        ("dk", dk, km, "tensor_sub")]
                if not direct:
                    plan = [("eq", eq, qm, "tensor_add"),
                            ("ek", ek, km, "tensor_add")] + plan
                for tag, lst, x, fn in plan:
                    for j in range(HB):
                        t = edp.tile([128, R], store, tag=tag)
                        getattr(nc.vector, fn)(t[:], x[:, j, :],
                                               x[:, j + 4, :])
                        lst.append(t)

                def chain(ps_slice, name_i, mb, xs, mega):
                    if xs is None:
                        for j in range(LB):
                            mm(ps_slice, fwd_st(name_i, j % HB, mb),
                               mega[:, j, :], j == 0, j == LB - 1)
                    else:
                        for j in range(HB):
                            mm(ps_slice, fwd_st(name_i, j, mb), xs[j][:],
                               j == 0, j == HB - 1)

                pre_sb, pim_sb = [], []
                eq_ = None if direct else eq
                ek_ = None if direct else ek
                groups = [(0, 1, eq_, ek_, 0), (0, 1, eq_, ek_, 1),
                          (2, 3, dq, dk, 0), (2, 3, dq, dk, 1)]
                for gi, (ma, mb_, xq, xk, mb) in enumerate(groups):
                    ps_q = psq.tile([128, 1024], _dt.float32, tag="fq")
                    ps_k = psk.tile([128, 1024], _dt.float32, tag="fk")
                    chain(ps_q[:, 0:R], ma, mb, xq, qm)
                    chain(ps_q[:, R:2 * R], mb_, mb, xq, qm)
                    q_sb = stp.tile([128, 1024], store, tag="qsb")
                    nc.scalar.mul(q_sb[:], ps_q[:], 1.0)
                    chain(ps_k[:, 0:R], ma, mb, xk, km)
                    chain(ps_k[:, R:2 * R], mb_, mb, xk, km)
                    k_sb = stp.tile([128, 1024], store, tag="ksb")
                    nc.scalar.mul(k_sb[:], ps_k[:], 1.0)

                    qre, qim = q_sb[:, 0:R], q_sb[:, R:2 * R]
                    kre, kim = k_sb[:, 0:R], k_sb[:, R:2 * R]
                    t1 = tp.tile([128, R], store, tag="t1")
                    t2 = tp.tile([128, R], store, tag="t2")
                    nc.vector.tensor_mul(t1[:], qre, kre)
                    nc.vector.tensor_mul(t2[:], qim, kim)
                    pre = pp.tile([128, R], store, tag="pre")
                    nc.vector.tensor_add(pre[:], t1[:], t2[:])
                    t3 = tp.tile([128, R], store, tag="t3")
                    t4 = tp.tile([128, R], store, tag="t4")
                    nc.vector.tensor_mul(t3[:], qim, kre)
                    nc.vector.tensor_mul(t4[:], qre, kim)
                    pim = pp.tile([128, R], store, tag="pim")
                    nc.vector.tensor_sub(pim[:], t3[:], t4[:])
                    if gi == 0:
                        # slot 0 packs DC (re) / Nyquist (im): overwrite
                        # with the pure products
                        nc.vector.tensor_copy(pre[0:1, :], t1[0:1, :])
                        nc.vector.tensor_copy(pim[0:1, :], t2[0:1, :])
                    pre_sb.append(pre)
                    pim_sb.append(pim)
                state[n] = (pre_sb, pim_sb)

            def inverse(n):
                pre_sb, pim_sb = state[n]
                acc = op.tile([128, 8], _dt.float32, tag="acc")
                accs[n] = acc
                cm = op.tile([128, LB, R], store, tag="cm")
                for tb in range(HB):
                    ps_uw = psi.tile([128, 1024], _dt.float32, tag="inv")
                    u, w = ps_uw[:, 0:R], ps_uw[:, R:2 * R]
                    mm(u, inv_st(0, 0, tb), pre_sb[0][:], True, False)
                    mm(u, inv_st(0, 1, tb), pre_sb[1][:], False, False)
                    mm(u, inv_st(1, 0, tb), pim_sb[0][:], False, False)
                    mm(u, inv_st(1, 1, tb), pim_sb[1][:], False, True)
                    mm(w, inv_st(2, 0, tb), pre_sb[2][:], True, False)
                    mm(w, inv_st(2, 1, tb), pre_sb[3][:], False, False)
                    mm(w, inv_st(3, 0, tb), pim_sb[2][:], False, False)
                    mm(w, inv_st(3, 1, tb), pim_sb[3][:], False, True)
                    uw_sb = uwp.tile([128, 1024], store, tag="uwsb")
                    # one staging copy per tb; its accumulator collects
                    # sum_r(u)+sum_r(w) = row-sums of corr_lo (the topk
                    # stat), the corr_hi STT's accumulator collects
                    # sum_r(u-w) = row-sums of corr_hi
                    nc.scalar.activation(
                        uw_sb[:], ps_uw[:],
                        mybir.ActivationFunctionType.Copy,
                        bias=0.0, scale=1.0, accum_out=acc[:, tb:tb + 1])
                    nc.vector.tensor_add(cm[:, tb, :], uw_sb[:, 0:R],
                                         uw_sb[:, R:2 * R])
                    nc.vector.scalar_tensor_tensor(
                        cm[:, tb + HB, :], uw_sb[:, 0:R], 1.0,
                        uw_sb[:, R:2 * R],
                        op0=mybir.AluOpType.mult,
                        op1=mybir.AluOpType.subtract,
                        accum_out=acc[:, 4 + tb:5 + tb])
                    # stream corr out per tb (blocks {tb, tb+4}) so the
                    # final batch leaves only a quarter-DMA tail
                    eng = nc.sync if (n + tb) % 2 == 0 else nc.gpsimd
                    eng.dma_start(rearr(corr_d[n])[:, tb:tb + 5:4, :],
                                  cm[:, tb:tb + 5:4, :])
                nc.gpsimd.dma_start(pacc_d[n][:], acc[:])

            # software pipeline: fwd(0), fwd(1), inv(0), fwd(2), inv(1), ...
            forward(0)
            for n in range(1, NLOC):
                forward(n)
                inverse(n - 1)
            inverse(NLOC - 1)
    nc.compile()
    return nc


def _build_phase2(entries, fuse_hi):
    """entries: per output block b, list of (src_block, seg_idx); seg_idx
    indexes the g stationaries tensor (NLOC, 128, nseg*128). fuse_hi: the
    block offset of the remainder-0 tap folded into the out-copy STT (its
    per-partition weight comes from the wv input; 0/w=0 when unused)."""
    nseg = max((si for segs in entries for _, si in segs), default=-1) + 1
    nseg = max(nseg, 1)
    nc = bacc.Bacc("TRN2", target_bir_lowering=False, debug=False,
                   num_devices=NCORES)
    v_d = nc.dram_tensor("v", [NLOC, L, R], _dt.bfloat16,
                         kind="ExternalInput").ap()
    g_d = nc.dram_tensor("g", [128, NLOC * nseg * 128], _dt.bfloat16,
                         kind="ExternalInput").ap()
    wv_d = nc.dram_tensor("wv", [128, NLOC], _dt.float32,
                          kind="ExternalInput").ap()
    out_d = nc.dram_tensor("out", [NLOC, L, R], _dt.bfloat16,
                           kind="ExternalOutput").ap()

    def rearr(ap):
        return ap.rearrange("(j p) r -> p j r", p=128)

    with tile.TileContext(nc) as tc:
        with tc.tile_pool(name="v", bufs=3) as vp, \
             tc.tile_pool(name="g", bufs=NLOC) as gp, \
             tc.tile_pool(name="o", bufs=2) as op, \
             tc.tile_pool(name="ps", bufs=8, space="PSUM") as psp:
            v_sb = []
            half = NLOC * nseg * 64
            g_sb = gp.tile([128, NLOC * nseg * 128], _dt.bfloat16, tag="g")
            nc.sync.dma_start(g_sb[:, 0:half], g_d[:, 0:half])
            wv_sb = gp.tile([128, NLOC], _dt.float32, tag="wv")
            nc.gpsimd.dma_start(wv_sb[:], wv_d[:])
            t = vp.tile([128, LB, R], _dt.bfloat16, tag="v")
            vr = rearr(v_d[0])
            nc.scalar.dma_start(t[:, 1:6:4, :], vr[:, 1:6:4, :])
            nc.gpsimd.dma_start(t[:, 2:7:4, :], vr[:, 2:7:4, :])
            nc.scalar.dma_start(t[:, 0:5:4, :], vr[:, 0:5:4, :])
            nc.sync.dma_start(t[:, 3:8:4, :], vr[:, 3:8:4, :])
            nc.scalar.dma_start(g_sb[:, half:2 * half], g_d[:, half:2 * half])
            v_sb.append(t)
            for n in range(NLOC):
                if n + 1 < NLOC:
                    t = vp.tile([128, LB, R], _dt.bfloat16, tag="v")
                    eng = (nc.gpsimd, nc.sync, nc.scalar)[n % 3]
                    eng.dma_start(t[:], rearr(v_d[n + 1]))
                    v_sb.append(t)
                om = op.tile([128, LB, R], _dt.bfloat16, tag="o")
                for b in range(LB):
                    segs = entries[b]
                    ps = psp.tile([128, R], _dt.float32, tag="ps")
                    for i, (a, si) in enumerate(segs):
                        nc.tensor.matmul(
                            ps[:], g_sb[:, ts(n * nseg + si, 128)],
                            v_sb[n][:, a, :],
                            start=(i == 0),
                            stop=(i == len(segs) - 1))
                    # fused remainder-0 tap: om = v[b+hi]*wv + psum
                    nc.vector.scalar_tensor_tensor(
                        om[:, b, :], v_sb[n][:, (b + fuse_hi) % LB, :],
                        wv_sb[:, n:n + 1], ps[:],
                        op0=mybir.AluOpType.mult, op1=mybir.AluOpType.add)
                    if b % 2:
                        eng = nc.sync if (n + b) % 4 < 2 else nc.scalar
                        eng.dma_start(rearr(out_d[n])[:, b - 1:b + 1, :],
                                      om[:, b - 1:b + 1, :])
    nc.compile()
    return nc


_P1_CACHE = {}


def _phase1_nc():
    if "p1" not in _P1_CACHE:
        _P1_CACHE["p1"] = _build_phase1()
    return _P1_CACHE["p1"]


def _run(nc, in_maps, phase):
    res = run_bass_kernel_spmd(nc, in_maps, core_ids=list(range(NCORES)),
                               trace=TRACE[0])
    if TRACE[0]:
        LAST_EXEC_NS[phase] = res.exec_time_ns
    return res.results


def kernel(queries, keys, values):
    queries = np.ascontiguousarray(np.asarray(queries, dtype=np.float32))
    keys = np.ascontiguousarray(np.asarray(keys, dtype=np.float32))
    values = np.ascontiguousarray(np.asarray(values, dtype=np.float32))

    store_np = ml_dtypes.bfloat16
    C5, S5, Mre, Mim, Au, Bu, Aw, Bw = _dft_mats()
    cf = np.concatenate([C5, S5, Mre, Mim], axis=1)   # [512, 1024]
    ci = np.concatenate([Au, Bu, Aw, Bw], axis=1)     # [256, 2048]
    cf = np.ascontiguousarray(cf.astype(np.float32)).astype(store_np)
    ci = np.ascontiguousarray(ci.astype(np.float32)).astype(store_np)

    q3 = queries.reshape(N, L, R)
    k3 = keys.reshape(N, L, R)
    v3 = values.reshape(N, L, R)

    nc1 = _phase1_nc()
    in_maps = []
    for c in range(NCORES):
        sl = slice(c * NLOC, (c + 1) * NLOC)
        in_maps.append({
            "q": q3[sl].astype(store_np),
            "k": k3[sl].astype(store_np),
            "cf": cf, "ci": ci,
        })
    res1 = _run(nc1, in_maps, 0)

    corr = np.concatenate([r["corr"] for r in res1], axis=0)  # (N, L, R)
    pacc = np.concatenate([r["pacc"] for r in res1], axis=0)  # (N, 128, 8)
    # stat: cols tb = sum_r corr_lo, cols 4+tb = sum_r corr_hi
    pacc = pacc.astype(np.float64)
    slo = pacc[:, :, 0:4].transpose(0, 2, 1).reshape(N, 512)
    shi = pacc[:, :, 4:8].transpose(0, 2, 1).reshape(N, 512)
    mean = np.concatenate([slo, shi], axis=1) / R               # (N, L)

    g = mean.mean(axis=0)
    idx = np.argsort(-g, kind="stable")[:TOPK]
    w = mean[:, idx]
    e = np.exp(w - w.max(axis=1, keepdims=True))
    w = (e / e.sum(axis=1, keepdims=True)).astype(np.float32)  # (N, TOPK)

    # phase-2 stationaries: out[b*128+j] += w_k * v[(b*128+j+idx_k) mod L].
    # One remainder-0 tap is folded into the out-copy STT (fuse_hi / wv);
    # the rest are merged per (b, src_block) into banded stationaries,
    # deduped across b (matrix content is b-independent).
    # among remainder-0 taps, fuse the one whose removal shrinks the
    # source-block union the most (ties broken by index order)
    def _union_without(fk):
        s = set()
        for kk in range(TOPK):
            if kk == fk:
                continue
            sh = int(idx[kk])
            s.add((sh // 128) % LB)
            if sh % 128 > 0:
                s.add((sh // 128 + 1) % LB)
        return len(s)

    r0taps = [kk for kk in range(TOPK) if int(idx[kk]) % 128 == 0]
    fuse_k = min(r0taps, key=_union_without) if r0taps else None
    fuse_hi = (int(idx[fuse_k]) // 128) % LB if fuse_k is not None else 0
    wv = (w[:, fuse_k] if fuse_k is not None
          else np.zeros(N, np.float32)).astype(np.float32)     # (N,)

    seg_of = {}
    pat = []
    entries = [[] for _ in range(LB)]
    for b in range(LB):
        acc = {}
        for kk in range(TOPK):
            if kk == fuse_k:
                continue
            sh = int(idx[kk])
            r = sh % 128
            a = ((b * 128 + sh) // 128) % LB
            acc.setdefault(a, []).append(("d1", r, kk))
            if r > 0:
                acc.setdefault((a + 1) % LB, []).append(("d2", r, kk))
        for a, parts in sorted(acc.items()):
            key = tuple(sorted(parts))
            if key not in seg_of:
                seg_of[key] = len(pat)
                pat.append(parts)
            entries[b].append((a, seg_of[key]))
    nseg = max(len(pat), 1)
    gmat = np.zeros((NLOC * NCORES, nseg, 128, 128), np.float32)
    jj = np.arange(128)
    for si, parts in enumerate(pat):
        for which, r, kk in parts:
            if which == "d1":
                j = jj[: 128 - r]
                gmat[:, si, j + r, j] += w[:, kk][:, None]
            else:
                j = jj[128 - r:]
                gmat[:, si, j - (128 - r), j] += w[:, kk][:, None]

    nc2 = _build_phase2(entries, fuse_hi)
    in_maps2 = []
    for c in range(NCORES):
        sl = slice(c * NLOC, (c + 1) * NLOC)
        # g packed as [128, NLOC*nseg*128]: slice (n*nseg+si) = gmat[n, si]
        gp_ = np.ascontiguousarray(
            gmat[sl].transpose(2, 0, 1, 3).reshape(128, NLOC * nseg * 128)
        ).astype(ml_dtypes.bfloat16)
        wv_ = np.ascontiguousarray(
            np.broadcast_to(wv[sl][None, :], (128, NLOC))).astype(np.float32)
        in_maps2.append({
            "v": v3[sl].astype(ml_dtypes.bfloat16),
            "g": gp_,
            "wv": wv_,
        })
    res2 = _run(nc2, in_maps2, 1)
    out = np.concatenate([np.asarray(r["out"], dtype=np.float32)
                          for r in res2], axis=0)             # (N, L, R)

    out_full = out.reshape(N, L, H, E).astype(np.float32)
    corr_full = corr.reshape(N, L, H, E).astype(np.float32)
    return out_full, corr_full
